# revision 6
# baseline (speedup 1.0000x reference)
"""GCN (2x GCNConv + graclus-style max-pool head) on 8 Trainium2 NeuronCores.

v4 design. Per-core 12500 nodes in 196 blocks of 64 (pairs kept together).
  - Layer 1 has NO gather: x rows are host-permuted into edge order (xg) and
    streamed; matmul associativity lets us aggregate raw 128-dim features
    with one-hot sel matmuls (agg = sel @ xg per block), then transform
    per 128-tile: h1 = aggT.T @ W1 + b1, tab2 = dinv * (h1 @ W2).
  - Sel blocks are 64 wide (128B/edge instead of 512B): 1 matmul per chunk,
    psum [64, .] per block.
  - Layer 2 gathers tab2 rows (bf16, 256B padded) with dma_gather; the 4
    table shards map to the 4 SWDGE queues so descriptor generation runs
    on all 4 Q7 core pairs concurrently (~4x the single-queue rate).
  - Self-loops: L1 folds host-precomputed dinv^2*x into agg; L2 folds
    dinv*tab2 (tab2 rows carry dinv[src]).
  - Pooling: first cluster per graph, one small dma_gather + max.
"""

import sys

sys.path.insert(0, "/opt/trn_rl_repo")

import numpy as np

N = 100000
E = 1600000
B = 256
IN_DIM = 128
OUT_DIM = 64
NCORES = 8
NS = N // NCORES          # 12500 real nodes per core
W = 64                    # dst block width
NB = (NS + W - 1) // W    # 196 blocks per core
NSP = NB * W              # 12544 padded nodes per core
NT = NSP // 128           # 98 tiles (for tab2/h2 IO)
GB = 4                    # blocks per group
NG = NB // GB             # 49 groups
NSH = 4                   # src table shards / SWDGE queues
# table shards = local-row quarters (tile-aligned) for pipelined AllGather
OFFQ = [0, 3200, 6400, 9472, 12544]
QSZ = [3200, 3200, 3072, 3072]
AGB = [12, 24, 36, 48]    # L1 group after which quarter q is complete
TOTR = NCORES * NSP       # 100352 table rows
P = 128
D = OUT_DIM
ROWW = 128                # padded table row width (elems); 256B in bf16


def _align16(x):
    return (x + 15) // 16 * 16


def _prepare(inputs):
    import ml_dtypes
    bf16 = ml_dtypes.bfloat16

    x = np.asarray(inputs["x"], dtype=np.float32)
    edge_index = np.asarray(inputs["edge_index"], dtype=np.int64)
    edge_weight = np.asarray(inputs["edge_weight"], dtype=np.float32)
    batch = np.asarray(inputs["batch"], dtype=np.int64)
    W1 = np.asarray(inputs["W1"], dtype=np.float32)
    b1 = np.asarray(inputs["b1"], dtype=np.float32)
    W2 = np.asarray(inputs["W2"], dtype=np.float32)
    b2 = np.asarray(inputs["b2"], dtype=np.float32)

    src = edge_index[0]
    dst = edge_index[1]
    w = edge_weight

    deg = np.bincount(dst, weights=w, minlength=N).astype(np.float32) + 1.0
    dinv = (1.0 / np.sqrt(deg)).astype(np.float32)

    # ---- degree-balanced node placement (pairs into 8*196 blocks) --------
    npairs = N // 2
    pdeg = np.bincount(dst // 2, minlength=npairs)
    order_p = np.argsort(-pdeg, kind="stable")
    nbuck = NCORES * NB
    cap = W // 2                       # 32 pairs per block
    bucket_of_pair = np.empty(npairs, np.int64)
    rank_in_bucket = np.empty(npairs, np.int64)
    fill = np.zeros(nbuck, np.int64)
    bi = 0
    direction = 1
    for pp in order_p:
        tries = 0
        while fill[bi] >= cap:
            bi += direction
            if bi == nbuck or bi < 0:
                direction = -direction
                bi += direction
            tries += 1
            assert tries <= 2 * nbuck
        bucket_of_pair[pp] = bi
        rank_in_bucket[pp] = fill[bi]
        fill[bi] += 1
        bi += direction
        if bi == nbuck or bi < 0:
            direction = -direction
            bi += direction
    pair_core = bucket_of_pair // NB
    pair_block = bucket_of_pair % NB
    node_core = np.repeat(pair_core, 2)
    node_local = (np.repeat(pair_block * W + 2 * rank_in_bucket, 2)
                  + np.tile(np.array([0, 1]), npairs)).astype(np.int64)
    # ----------------------------------------------------------------------

    ecore = node_core[dst]
    elocal = node_local[dst]
    eb = elocal // W                  # dst block 0..195
    edl = elocal % W                  # dst lane in block
    sl = node_local[src]
    es = np.searchsorted(np.asarray(OFFQ), sl, side="right") - 1  # quarter
    qsz = np.asarray(QSZ)[es]
    offq = np.asarray(OFFQ)[es]
    eli = (node_core[src] * qsz + (sl - offq)).astype(np.int16)
    norm1 = (dinv[src] * w * dinv[dst]).astype(np.float32)   # L1 sel weight
    wsel2 = (w * dinv[dst]).astype(np.float32)               # L2 sel weight

    # ---- L1 cells: per block ---------------------------------------------
    cnt1 = np.zeros((NCORES, NB), np.int64)
    np.add.at(cnt1, (ecore, eb), 1)
    K1 = (cnt1.max(axis=0) + P - 1) // P            # chunks per block
    cb1 = np.concatenate([[0], np.cumsum(K1)])      # chunk base per block
    TC1 = int(cb1[-1])

    # ---- L2 cells: per (block, shard), grouped by 4 blocks ---------------
    cnt2 = np.zeros((NCORES, NB, NSH), np.int64)
    np.add.at(cnt2, (ecore, eb, es), 1)
    sub = _align16(cnt2.max(axis=0))                # [NB, NSH] rows per (b,s)
    # within cell (g, s): block offsets
    sub_g = sub.reshape(NG, GB, NSH)
    off_in_cell = np.zeros((NG, GB, NSH), np.int64)
    for bi_ in range(1, GB):
        off_in_cell[:, bi_, :] = off_in_cell[:, bi_ - 1, :] + sub_g[:, bi_ - 1, :]
    NI2 = sub_g.sum(axis=1)                         # [NG, NSH] rows per cell
    Kcell = (NI2 + P - 1) // P                      # chunks per cell
    cbase = np.zeros((NG, NSH), np.int64)           # chunk base within group
    Cg2 = np.zeros(NG, np.int64)
    for g in range(NG):
        o = 0
        for s in range(NSH):
            cbase[g, s] = o
            o += int(Kcell[g, s])
        Cg2[g] = o
    icb2 = np.zeros((NG, NSH), np.int64)            # idx col base
    run = 0
    for g in range(NG):
        for s in range(NSH):
            icb2[g, s] = run
            run += int(NI2[g, s]) // 16
    ICOLS2 = int(run)

    # ---- L2 matmul plan (shared across cores) ----------------------------
    # per group: list of (block_in_group, shard, chunk_in_group, mmslot,
    #                     start, stop); ordered per block by (s, chunk).
    mm_plan = []          # list per group
    mmslot_of = {}        # (g, bi, s, cp_in_cell) -> mmslot
    mtot = 0
    for g in range(NG):
        entries = []      # (bi, s, cp_in_cell)
        for bi_ in range(GB):
            for s in range(NSH):
                o0 = int(off_in_cell[g, bi_, s])
                o1 = o0 + int(sub_g[g, bi_, s])
                if o1 == o0:
                    continue
                for cp in range(o0 // P, (o1 + P - 1) // P):
                    entries.append((bi_, s, cp))
        entries.sort()
        plan = []
        for k, (bi_, s, cp) in enumerate(entries):
            mmslot_of[(g, bi_, s, cp)] = mtot + k
            start = k == 0 or entries[k - 1][0] != bi_
            stop = k == len(entries) - 1 or entries[k + 1][0] != bi_
            plan.append((bi_, s, int(cbase[g, s]) + cp, mtot + k, start, stop))
        mm_plan.append(plan)
        mtot += len(plan)
    MM2 = mtot

    # pooling: first cluster per graph
    ncl = N // 2
    bp = batch[0::2]
    first = np.full(B, np.iinfo(np.int32).max, np.int64)
    np.minimum.at(first, bp, np.arange(ncl, dtype=np.int64))
    cl = np.clip(first, 0, ncl - 1)
    row_even = 2 * cl
    owner = node_core[row_even]
    loc_even = node_local[row_even]

    b1r = np.broadcast_to(b1, (P, D)).copy()
    b2r = np.broadcast_to(b2, (P, D)).copy()

    in_maps = []
    for c in range(NCORES):
        m = ecore == c
        ebm = eb[m]
        esm = es[m]
        edlm = edl[m]
        elim = eli[m]
        n1m = norm1[m]
        w2m = wsel2[m]
        srcm = src[m]

        # ---- L1 streams: xg + sel1, slot = 128*cb1[b] + rank -------------
        order1 = np.argsort(ebm, kind="stable")
        eb1 = ebm[order1]
        starts = np.concatenate([[0], np.cumsum(np.bincount(eb1, minlength=NB))])[:-1]
        rank1 = np.arange(eb1.size) - starts[eb1]
        slot1 = P * cb1[eb1] + rank1
        lane1 = slot1 % P
        cp1 = slot1 // P
        xg = np.zeros((P, TC1 * IN_DIM), np.float32)
        xg[lane1[:, None], (cp1 * IN_DIM)[:, None] + np.arange(IN_DIM)[None, :]] = x[srcm[order1]]
        xg = xg.astype(bf16)
        sel1 = np.zeros((P, TC1 * W), np.float32)
        sel1[lane1, cp1 * W + edlm[order1]] = n1m[order1]
        sel1 = sel1.astype(bf16)

        # ---- L2: gather idx + sel2 ---------------------------------------
        # position within cell (g, s): off_in_cell[g,bi,s] + rank2
        gkey = (ebm * NSH + esm)
        order2 = np.argsort(gkey, kind="stable")
        k2 = gkey[order2]
        starts2 = np.concatenate([[0], np.cumsum(np.bincount(k2, minlength=NB * NSH))])[:-1]
        rank2 = np.arange(k2.size) - starts2[k2]
        b2_ = k2 // NSH
        s2_ = k2 % NSH
        g2_ = b2_ // GB
        bi2 = b2_ % GB
        pos = off_in_cell[g2_, bi2, s2_] + rank2          # row within cell
        # gather idx array: cell (g,s) wrapped int16 at col base icb2
        gidx_arr = np.zeros((P, ICOLS2), np.int16)
        col = icb2[g2_, s2_] + pos // 16
        row16 = pos % 16
        liv = elim[order2]
        for rep in range(8):
            gidx_arr[16 * rep + row16, col] = liv
        # sel2: lane = pos % 128, mmslot = mmslot_of[(g, bi, s, pos//128)]
        cpc = pos // P
        mslot = np.empty(pos.size, np.int64)
        for i_ in range(pos.size):
            mslot[i_] = mmslot_of[(g2_[i_], bi2[i_], s2_[i_], cpc[i_])]
        sel2 = np.zeros((P, MM2 * W), np.float32)
        sel2[pos % P, mslot * W + edlm[order2]] = w2m[order2]
        sel2 = sel2.astype(bf16)

        # ---- per-core node-major data ------------------------------------
        mc = node_core == c
        locs = node_local[mc]
        # xNT = dinv^2 * x  (self-loop contribution to L1 agg),
        # feature-major: [IN_DIM, NSP] so it folds into aggT directly
        xNT = np.zeros((IN_DIM, NSP), np.float32)
        xNT[:, locs] = ((dinv[mc] ** 2)[:, None] * x[mc]).T
        xNT = xNT.astype(bf16)
        dinv_blk = np.zeros((W, NB), np.float32)
        dinv_blk[locs % W, locs // W] = dinv[mc]

        # pooling gather indices
        pe = np.where(owner == c, loc_even, 0).astype(np.int64)
        po = np.where(owner == c, loc_even + 1, 0).astype(np.int64)
        pidx_flat = np.concatenate([pe, po]).astype(np.int16)
        pidx_arr = np.zeros((P, 32), np.int16)
        jj = np.arange(512)
        for rep in range(8):
            pidx_arr[16 * rep + jj % 16, jj // 16] = pidx_flat

        in_maps.append({
            "xg": xg,
            "sel1": sel1,
            "xN": xNT,
            "gidx": gidx_arr,
            "sel2": sel2,
            "pidx": pidx_arr,
            "dinvb": dinv_blk,
            "W1": W1,
            "W2": W2,
            "b1c": b1.reshape(D, 1).copy(),
            "b2r": b2r,
        })

    tables = dict(K1=K1, cb1=cb1, TC1=TC1, NI2=NI2, Kcell=Kcell, cbase=cbase,
                  Cg2=Cg2, icb2=icb2, ICOLS2=ICOLS2, mm_plan=mm_plan, MM2=MM2)
    return in_maps, tables, owner


def _build(tables):
    import concourse.bass as bass
    import concourse.tile as tile
    from concourse import mybir, bacc, library_config

    K1 = tables["K1"]
    cb1 = tables["cb1"]
    TC1 = tables["TC1"]
    NI2 = tables["NI2"]
    Kcell = tables["Kcell"]
    cbase = tables["cbase"]
    Cg2 = tables["Cg2"]
    icb2 = tables["icb2"]
    ICOLS2 = tables["ICOLS2"]
    mm_plan = tables["mm_plan"]
    MM2 = tables["MM2"]

    f32 = mybir.dt.float32
    bf = mybir.dt.bfloat16
    i16 = mybir.dt.int16
    AOP = mybir.AluOpType

    nc = bacc.Bacc("TRN2", target_bir_lowering=False, debug=False,
                   num_devices=NCORES, dynamic_dma_scratch_size=32768,
                   num_swdge_queues=4)

    xg = nc.declare_dram_parameter("xg", [P, TC1 * IN_DIM], bf, isOutput=False)
    sel1 = nc.declare_dram_parameter("sel1", [P, TC1 * W], bf, isOutput=False)
    xN = nc.declare_dram_parameter("xN", [IN_DIM, NSP], bf, isOutput=False)
    gidx = nc.declare_dram_parameter("gidx", [P, ICOLS2], i16, isOutput=False)
    sel2 = nc.declare_dram_parameter("sel2", [P, MM2 * W], bf, isOutput=False)
    pidx = nc.declare_dram_parameter("pidx", [P, 32], i16, isOutput=False)
    dinvb = nc.declare_dram_parameter("dinvb", [W, NB], f32, isOutput=False)
    W1 = nc.declare_dram_parameter("W1", [IN_DIM, D], f32, isOutput=False)
    W2 = nc.declare_dram_parameter("W2", [D, D], f32, isOutput=False)
    b1c = nc.declare_dram_parameter("b1c", [D, 1], f32, isOutput=False)
    b2r = nc.declare_dram_parameter("b2r", [P, D], f32, isOutput=False)
    pool_out = nc.declare_dram_parameter("pool_out", [P, 2, D], f32, isOutput=True)

    tab2_mine = nc.dram_tensor("tab2_mine", [NSP, ROWW], bf)
    tab2q = [nc.dram_tensor(f"tab2q{q}", [NCORES * QSZ[q], ROWW], bf,
                            addr_space="Shared") for q in range(NSH)]
    h2_local = nc.dram_tensor("h2_local", [NSP, D], f32)

    groups = [list(range(NCORES))]

    from contextlib import ExitStack
    with ExitStack() as top:
        tc = top.enter_context(tile.TileContext(nc))
        nc.gpsimd.load_library(library_config.mlp)
        const = top.enter_context(tc.tile_pool(name="const", bufs=1))
        W1_t = const.tile([IN_DIM, D], f32)
        nc.sync.dma_start(out=W1_t[:], in_=W1[:])
        W2_t = const.tile([D, D], f32)
        nc.sync.dma_start(out=W2_t[:], in_=W2[:])
        b1c_t = const.tile([D, 1], f32)
        nc.sync.dma_start(out=b1c_t[:], in_=b1c[:])
        b2r_t = const.tile([P, D], f32)
        nc.sync.dma_start(out=b2r_t[:], in_=b2r[:])
        dinvb_t = const.tile([W, NB], f32)
        nc.sync.dma_start(out=dinvb_t[:], in_=dinvb[:])
        from concourse.masks import make_identity
        ident = const.tile([P, P], f32)
        make_identity(nc, ident[:])

        tab2_blk = const.tile([W, NB, D], bf)
        gidx_t = const.tile([P, ICOLS2], i16)
        nc.sync.dma_start(out=gidx_t[:], in_=gidx[:])

        # ================= layer 1: sel1 @ xg, transform ==================
        with tc.tile_pool(name="l1", bufs=3) as l1p, \
             tc.tile_pool(name="l1f", bufs=3) as l1f, \
             tc.tile_pool(name="l1ps", bufs=2, space="PSUM") as l1ps, \
             tc.tile_pool(name="l1ps2", bufs=2, space="PSUM") as l1ps2:
            for g in range(NG):
                c0 = int(cb1[g * GB])
                c1 = int(cb1[(g + 1) * GB])
                kg = c1 - c0
                xg_t = l1p.tile([P, kg * IN_DIM], bf, tag="xg")
                nc.sync.dma_start(out=xg_t[:],
                                  in_=xg[:, c0 * IN_DIM:c1 * IN_DIM])
                sel1_t = l1p.tile([P, kg * W], bf, tag="sel1")
                nc.scalar.dma_start(out=sel1_t[:], in_=sel1[:, c0 * W:c1 * W])
                xNT_t = l1p.tile([IN_DIM, GB * W], bf, tag="xNT")
                nc.sync.dma_start(
                    out=xNT_t[:],
                    in_=xN[:, g * GB * W:(g + 1) * GB * W])
                tab2t_g = l1f.tile([W, GB, D], bf, tag="tab2t")
                for bi_ in range(GB):
                    b = g * GB + bi_
                    kb = int(K1[b])
                    cb = int(cb1[b]) - c0
                    # aggT = sum_chunks xg_chunk.T @ sel_chunk  [128f, 64n]
                    ps = l1ps.tile([IN_DIM, W], f32, tag="agg_ps")
                    for j in range(kb):
                        nc.tensor.matmul(
                            out=ps[:],
                            lhsT=xg_t[:, (cb + j) * IN_DIM:(cb + j + 1) * IN_DIM],
                            rhs=sel1_t[:, (cb + j) * W:(cb + j + 1) * W],
                            start=(j == 0), stop=(j == kb - 1))
                    aggT = l1f.tile([IN_DIM, W], f32, tag="aggT")
                    nc.vector.tensor_tensor(
                        out=aggT[:], in0=ps[:],
                        in1=xNT_t[:, bi_ * W:(bi_ + 1) * W], op=AOP.add)
                    # h1T = W1.T @ aggT + b1 (bias per partition)
                    psH = l1ps.tile([D, W], f32, tag="mmout")
                    nc.tensor.matmul(out=psH[:], lhsT=W1_t[:], rhs=aggT[:],
                                     start=True, stop=True)
                    h1T = l1f.tile([D, W], f32, tag="h1T")
                    nc.scalar.add(out=h1T[:], in_=psH[:], add=b1c_t[:, 0:1])
                    # tab2T = W2.T @ h1T, transpose back, scale by dinv
                    psW2 = l1ps.tile([D, W], f32, tag="mmout")
                    nc.tensor.matmul(out=psW2[:], lhsT=W2_t[:], rhs=h1T[:],
                                     start=True, stop=True)
                    t2T = l1f.tile([D, W], f32, tag="t2T")
                    nc.vector.tensor_copy(out=t2T[:], in_=psW2[:])
                    psN = l1ps2.tile([W, D], f32, tag="psN")
                    nc.tensor.transpose(out=psN[:], in_=t2T[:],
                                        identity=ident[0:D, 0:D])
                    nc.vector.tensor_scalar_mul(out=tab2t_g[:, bi_, :],
                                                in0=psN[:],
                                                scalar1=dinvb_t[:, b:b + 1])
                    nc.vector.tensor_copy(out=tab2_blk[:, b, :],
                                          in_=tab2t_g[:, bi_, :])
                nc.sync.dma_start(
                    out=tab2_mine[g * GB * W:(g + 1) * GB * W, 0:D].rearrange(
                        "(bi lane) d -> lane bi d", bi=GB),
                    in_=tab2t_g[:])
                for q in range(NSH):
                    if g == AGB[q]:
                        nc.gpsimd.collective_compute(
                            "AllGather", AOP.bypass, replica_groups=groups,
                            ins=[tab2_mine[OFFQ[q]:OFFQ[q + 1], :]],
                            outs=[tab2q[q][:]])

        # ================= layer 2: gather + sel2 matmuls =================
        CGMAX = int(Cg2.max())
        with tc.tile_pool(name="l2", bufs=4) as l2p, \
             tc.tile_pool(name="l2f", bufs=3) as l2f, \
             tc.tile_pool(name="l2ps", bufs=4, space="PSUM") as l2ps:
            ms0 = 0
            for g in range(NG):
                mmk = len(mm_plan[g])
                sel2_t = l2p.tile([P, mmk * W], bf, tag="sel2")
                nc.scalar.dma_start(out=sel2_t[:],
                                    in_=sel2[:, ms0 * W:(ms0 + mmk) * W])
                # fixed-size gbuf: memset each buffer once (first 6 groups);
                # later tail garbage is stale finite bf16 and sel rows are 0
                gbuf = l2p.tile([P, CGMAX, ROWW], bf, tag="gbuf", bufs=6)
                if g < 6:
                    nc.vector.memset(gbuf[:], 0.0)
                for s in range(NSH):
                    ni = int(NI2[g, s])
                    if ni == 0:
                        continue
                    cb = int(cbase[g, s])
                    nk = int(Kcell[g, s])
                    il0 = int(icb2[g, s])
                    nc.gpsimd.dma_gather(
                        gbuf[:, cb:cb + nk, :],
                        tab2q[s][:],
                        gidx_t[:, il0:il0 + ni // 16],
                        ni, ni, ROWW, single_packet=False, queue_num=s)
                selfw = l2f.tile([W, GB, D], f32, tag="selfw")
                for bi_ in range(GB):
                    b = g * GB + bi_
                    nc.scalar.mul(out=selfw[:, bi_, :], in_=tab2_blk[:, b, :],
                                  mul=dinvb_t[:, b:b + 1])
                h2g = l2f.tile([W, GB, D], f32, tag="h2g")
                pss = {}
                for bi_, s, cp, mslot, st, sp in mm_plan[g]:
                    if st:
                        pss[bi_] = l2ps.tile([W, D], f32, tag="eps",
                                             name=f"eps{bi_}")
                    nc.tensor.matmul(
                        out=pss[bi_][:],
                        lhsT=sel2_t[:, (mslot - ms0) * W:(mslot - ms0 + 1) * W],
                        rhs=gbuf[:, cp, 0:D],
                        start=st, stop=sp)
                    if sp:
                        outb = l2f.tile([W, D], f32, tag="outb")
                        nc.vector.tensor_tensor(
                            out=outb[:], in0=pss[bi_][:],
                            in1=selfw[:, bi_, :], op=AOP.add)
                        nc.vector.tensor_tensor(
                            out=h2g[:, bi_, :], in0=outb[:], in1=b2r_t[0:W, :],
                            op=AOP.add)
                nc.sync.dma_start(
                    out=h2_local[g * GB * W:(g + 1) * GB * W, :].rearrange(
                        "(bi lane) d -> lane bi d", bi=GB),
                    in_=h2g[:])
                ms0 += mmk

        # ================= pooling head ===================================
        with tc.tile_pool(name="poolp", bufs=1) as pp:
            pidx_t = pp.tile([P, 32], i16)
            nc.sync.dma_start(out=pidx_t[:], in_=pidx[:])
            pbuf = pp.tile([P, 4, D], f32)
            nc.gpsimd.dma_gather(pbuf[:], h2_local[:], pidx_t[:], 512, 512, D,
                                 single_packet=False)
            pm = pp.tile([P, 2, D], f32)
            nc.vector.tensor_tensor(out=pm[:], in0=pbuf[:, 0:2, :],
                                    in1=pbuf[:, 2:4, :], op=AOP.max)
            nc.sync.dma_start(out=pool_out[:], in_=pm[:])

    nc.compile()
    return nc


LAST_RESULTS = None


def kernel(**inputs):
    global LAST_RESULTS
    from concourse.bass_utils import run_bass_kernel_spmd

    in_maps, tables, owner = _prepare(inputs)
    nc = _build(tables)
    res = run_bass_kernel_spmd(nc, in_maps, list(range(NCORES)))
    LAST_RESULTS = res
    out = np.zeros((B, D), np.float32)
    bb = np.arange(B)
    for c in range(NCORES):
        m = owner == c
        if m.any():
            po = res.results[c]["pool_out"]
            out[bb[m]] = po[bb[m] % P, bb[m] // P, :]
    return out
